# revision 31
# baseline (speedup 1.0000x reference)
"""Distributed causal multi-head attention with cumulative relative-position
bias for Trainium2 (8 NeuronCores).

Problem: x:[2,2048,1024], qkv:[1024,3,16,64], out_w:[16,64,1024],
rpe_bias:[16,2048] -> out:[2,2048,1024]

Sharding: data-parallel over batch (2) x tensor-parallel over head groups
(16 heads -> 4 groups of 4). Core c handles batch c//4, heads [4*(c%4), 4*(c%4)+4).
Each core emits a partial output [2048,1024] (bf16); the host sums the 4
head-group partials per batch (the "all-reduce" of the out projection).

v2 structure (vs v1 baseline):
 - exact causal banding: score/PV/exp/gmul bands start at k0 (128-aligned)
   instead of 512-aligned -> 15% less work in phase B.
 - head-PAIR processing with PE row tiling: the two heads of a pair live at
   partitions 0-63 / 64-127, so their K=64 score matmuls land in different
   PE row-groups (tile_position (0,0)/(64,0)) and run concurrently.
 - q processed in halves (lo=[0,1024), hi=[1024,2048)) so each head's mix
   accumulator is a [65,1024] PSUM tile; PSUM = sband(4 banks) + mixA(2) +
   mixB(2).
 - scores contract K=128 through zero-padded per-head K tiles (fast
   full-size ldweights path; the tile-positioned K=64 path measured ~1.8x
   slower per matmul).
 - PV matmuls and the softmax dances are deferred ~1 band via a pending
   queue so no engine queue head-blocks (keeps the PE p-state high).
 - softmax normalization: DVE evict of mix to SBUF (frees PSUM early),
   exact reciprocal via a [128,8] DMA-transposed layout, stride-0 DMA
   broadcast, all-SBUF normalize mul. (reciprocal_approx_fast is unusable:
   its custom-DVE lowering drops the input AP's base partition.)
 - exp owns the Act engine in phase B; phase-A evictions run on Act while
   it is otherwise idle; g-muls split DVE (wide) / GpSimd (narrow).
 - npair-1 projections and vproj st8-15 interleave into the q-lo attention
   bands as PE filler; out projection accumulates both head pairs in PSUM
   and stores bf16 (host sums the 4 head-group partials in f32).
"""

import sys

if "/opt/trn_rl_repo" not in sys.path:
    sys.path.insert(0, "/opt/trn_rl_repo")

import numpy as np
import ml_dtypes

B, S, HID, NH, D = 2, 2048, 1024, 16, 64
NCORES = 8
HPC = 4  # heads per core
KB = 16  # 128-row k blocks
BF16 = ml_dtypes.bfloat16

_CACHE = {}


def _bands(qhalf):
    """Causal bands for one q-half: (kb, qq, w) with qq = exact k0 start."""
    qbase = 1024 * qhalf
    qend = qbase + 1024
    out = []
    for kb in range(KB):
        k0 = 128 * kb
        qq = max(k0, qbase)
        if qq >= qend:
            continue
        out.append((kb, qq, qend - qq))
    return out


def build_nc():
    import concourse.mybir as mybir
    from concourse import bacc
    from concourse.tile import TileContext

    f32 = mybir.dt.float32
    bf16 = mybir.dt.bfloat16
    Exp = mybir.ActivationFunctionType.Exp
    Copy = mybir.ActivationFunctionType.Copy

    nc = bacc.Bacc()

    x_t = nc.declare_dram_parameter("x_t", [HID, S], bf16, isOutput=False)
    w_qk = nc.declare_dram_parameter("w_qk", [HID, 512], bf16, isOutput=False)
    w_v = nc.declare_dram_parameter("w_v", [HID, 260], bf16, isOutput=False)
    g_ext = nc.declare_dram_parameter("g_ext", [HPC, 128, 512 + S], bf16, isOutput=False)
    w_out = nc.declare_dram_parameter("w_out", [256, HID], bf16, isOutput=False)
    out = nc.declare_dram_parameter("out", [S, HID], bf16, isOutput=True)

    with TileContext(nc) as tc:
        with (
            tc.tile_pool(name="persist", bufs=1) as persist,
            tc.tile_pool(name="work", bufs=3) as work,
            tc.tile_pool(name="dram", bufs=2, space="DRAM") as dpool,
            tc.tile_pool(name="psum", bufs=2, space="PSUM") as psum,
        ):
            # ---------------- input DMAs ----------------
            # sync carries the critical path in arrival order: w_qk, x-lo,
            # x-hi. gpsimd queue: wv, wout, then g (first needed ~35us in).
            wqk_sb = []
            for i in range(8):
                t = persist.tile([128, 512], bf16, tag=f"wqk{i}", name=f"wqk{i}")
                nc.scalar.dma_start(out=t, in_=w_qk[i * 128 : (i + 1) * 128, :])
                wqk_sb.append(t)
            xt_sb = [
                persist.tile([128, S], bf16, tag=f"xt{i}", name=f"xt{i}")
                for i in range(8)
            ]
            for i in range(8):
                nc.sync.dma_start(
                    out=xt_sb[i][:, :1024], in_=x_t[i * 128 : (i + 1) * 128, :1024]
                )
            for i in range(8):
                nc.gpsimd.dma_start(
                    out=xt_sb[i][:, 1024:], in_=x_t[i * 128 : (i + 1) * 128, 1024:]
                )
            wv_sb = []
            for i in range(8):
                t = persist.tile([128, 260], bf16, tag=f"wv{i}", name=f"wv{i}")
                nc.gpsimd.dma_start(out=t, in_=w_v[i * 128 : (i + 1) * 128, :])
                wv_sb.append(t)
            g_sb = []
            for h in range(HPC):
                t = persist.tile([128, 512 + S], bf16, tag=f"g{h}", name=f"g{h}")
                nc.gpsimd.dma_start(out=t, in_=g_ext[h])
                g_sb.append(t)
            wout_sb = []
            for p in range(2):
                t = persist.tile([128, HID], bf16, tag=f"wout{p}", name=f"wout{p}")
                nc.gpsimd.dma_start(out=t, in_=w_out[p * 128 : (p + 1) * 128, :])
                wout_sb.append(t)

            # q tiles: [128, S], rows 0-63 head A, 64-127 head B (per pair).
            q_sb = [persist.tile([128, S], bf16, tag=f"q{p}", name=f"q{p}") for p in range(2)]
            # k tiles: zero-padded per head so score matmuls contract K=128
            # through the full-size (fast) ldweights path.
            ktp = {}
            for p in range(2):
                for hh in range(2):
                    ktp[(p, hh)] = persist.tile(
                        [128, S], bf16, tag=f"ktp{p}{hh}", name=f"ktp{p}{hh}"
                    )
            v_sb = [persist.tile([128, 260], bf16, tag=f"v{st}", name=f"v{st}") for st in range(KB)]
            mixT_sb = [
                [
                    persist.tile([128, 1024], bf16, tag=f"mixT{p}{qh}", name=f"mixT{p}{qh}")
                    for qh in range(2)
                ]
                for p in range(2)
            ]

            # Warm the Act Exp table during the DMA wait; zero the ktp pads.
            warm = work.tile([1, 16], f32, tag="warm", name="warm", bufs=1)
            nc.vector.memset(warm, 0.0)
            warm2 = work.tile([1, 16], f32, tag="warm2", name="warm2", bufs=1)
            nc.scalar.activation(out=warm2, in_=warm, func=Exp)
            for p in range(2):
                nc.vector.memset(ktp[(p, 0)][64:128, :], 0.0)
                nc.vector.memset(ktp[(p, 1)][0:64, :], 0.0)

            # ---------------- projection emitters ----------------
            def qkproj(npair, mt, half):
                """One [128,512] column chunk of the QK projection.
                mt 0/1 = q of pair 0/1, mt 2/3 = k of pair 0/1."""
                nq = npair * 2 + half
                ps = psum.tile([128, 1024], f32, tag="sb", name="qkps")[
                    :, half * 512 : (half + 1) * 512
                ]
                for xc in range(8):
                    nc.tensor.matmul(
                        ps,
                        lhsT=wqk_sb[xc][:, mt * 128 : (mt + 1) * 128],
                        rhs=xt_sb[xc][:, nq * 512 : (nq + 1) * 512],
                        start=(xc == 0),
                        stop=(xc == 7),
                    )
                cols = slice(nq * 512, (nq + 1) * 512)
                if mt < 2:
                    dst = q_sb[mt][:, cols]
                    nc.scalar.activation(out=dst, in_=ps, func=Copy)
                else:
                    p = mt - 2
                    nc.vector.tensor_copy(out=ktp[(p, 0)][0:64, cols], in_=ps[0:64, :])
                    nc.vector.tensor_copy(
                        out=ktp[(p, 1)][64:128, cols], in_=ps[64:128, :]
                    )

            def vproj(st):
                ps = psum.tile([128, 1024], f32, tag="sb", name="vps")[:, :260]
                for xc in range(8):
                    nc.tensor.matmul(
                        ps,
                        lhsT=xt_sb[xc][:, st * 128 : (st + 1) * 128],
                        rhs=wv_sb[xc],
                        start=(xc == 0),
                        stop=(xc == 7),
                    )
                nc.scalar.activation(out=v_sb[st], in_=ps, func=Copy)
                ones_cols = v_sb[st].rearrange("p (h c) -> p h c", c=65)
                nc.vector.memset(ones_cols[:, :, 64:65], 1.0)

            # upfront: everything attention-lo needs
            for mt in (0, 2, 1, 3):
                for half in range(2):
                    qkproj(0, mt, half)
            for st in range(8):
                vproj(st)
            # filler units run interleaved into the lo attention bands
            filler = []
            for mt in (0, 2, 1, 3):
                for half in range(2):
                    filler.append(lambda mt=mt, half=half: qkproj(1, mt, half))
            for st in range(8, KB):
                filler.append(lambda st=st: vproj(st))

            # ---------------- Phase B: attention ----------------
            last_kb = {0: (3, 7), 1: (11, 15)}
            groups = [(qh, p) for qh in range(2) for p in range(2)]
            pend = {}
            slot = 0

            def dance(mixs, p, qbase, hh, evict_eng=None):
                msb = work.tile([65, 1024], f32, tag="msb", name="msb", bufs=3)
                if evict_eng == "act":
                    nc.scalar.activation(out=msb, in_=mixs[hh][:65, :], func=Copy)
                else:
                    nc.vector.tensor_copy(out=msb, in_=mixs[hh][:65, :])

                def chain(p=p, qbase=qbase, hh=hh, msb=msb):
                    d1 = dpool.tile([1, 1024], f32, tag="d1", name="d1", bufs=3)
                    nc.sync.dma_start(out=d1, in_=msb[64:65, :])
                    rs = work.tile([128, 8], f32, tag="rs", name="rs", bufs=3)
                    nc.sync.dma_start(
                        out=rs, in_=d1.rearrange("o (a b) -> (o a) b", a=128)
                    )
                    rc = work.tile([128, 8], f32, tag="rc", name="rc", bufs=3)
                    nc.vector.reciprocal(out=rc, in_=rs)
                    d2 = dpool.tile([1, 1024], f32, tag="d2", name="d2", bufs=3)
                    nc.sync.dma_start(
                        out=d2.rearrange("o (a b) -> (o a) b", a=128), in_=rc
                    )
                    bc = work.tile([64, 1024], f32, tag="bc", name="bc", bufs=3)
                    nc.sync.dma_start(out=bc, in_=d2.to_broadcast([64, 1024]))

                    def mul(p=p, qbase=qbase, hh=hh, msb=msb, bc=bc):
                        eng = nc.gpsimd if qbase >= 1024 else nc.vector
                        eng.tensor_mul(
                            mixT_sb[p][qbase // 1024][64 * hh : 64 * hh + 64, :],
                            msb[0:64, :],
                            bc,
                        )

                    return mul

                return chain

            oq_count = [0]

            def outproj(qb, tag):
                i = oq_count[0]
                oq_count[0] += 1
                ps = psum.tile([128, 1024], f32, tag=tag, name="outps",
                               bufs=1 if tag != "sb" else None)
                for nn in range(2):
                    for pp in range(2):
                        nc.tensor.matmul(
                            ps[:, nn * 512 : (nn + 1) * 512],
                            lhsT=mixT_sb[pp][qb // 8][
                                :, (qb % 8) * 128 : (qb % 8 + 1) * 128
                            ],
                            rhs=wout_sb[pp][:, nn * 512 : (nn + 1) * 512],
                            start=(pp == 0),
                            stop=(pp == 1),
                        )
                osb = work.tile([128, HID], bf16, tag="osb", name="osb")
                if i % 2 == 0:
                    nc.vector.tensor_copy(out=osb, in_=ps)
                else:
                    nc.scalar.activation(out=osb, in_=ps, func=Copy)
                eng = nc.sync if i % 2 == 0 else nc.gpsimd
                eng.dma_start(out=out[qb * 128 : (qb + 1) * 128, :], in_=osb)

            for qhalf, p in groups:
                qbase = 1024 * qhalf
                bl = _bands(qhalf)
                mixs = [
                    psum.tile([128, 1024], f32, tag="mxA", name="mxA", bufs=1),
                    psum.tile([128, 1024], f32, tag="mxB", name="mxB", bufs=1),
                ]
                for bi, (kb, qq, w) in enumerate(bl):
                    k0 = 128 * kb
                    sbs, pbs = [], []
                    for hh in range(2):
                        sbs.append(
                            psum.tile([128, 1024], f32, tag="sb", name="sband")
                        )
                    # scores: zero-padded K=128 contraction, full ldweights
                    for hh in range(2):
                        kw = ktp[(p, hh)]
                        for c0 in range(0, w, 512):
                            cw = min(512, w - c0)
                            nc.tensor.matmul(
                                sbs[hh][:, c0 : c0 + cw],
                                lhsT=kw[:, k0 : k0 + 128],
                                rhs=q_sb[p][:, qq + c0 : qq + c0 + cw],
                                start=True,
                                stop=True,
                            )
                    for fn in pend.pop(slot, []):
                        fn()
                    gs = 512 + qq - k0
                    for hh in range(2):
                        pex = work.tile([128, 1024], bf16, tag="pex", name="pex", bufs=4)
                        nc.scalar.activation(
                            out=pex[:, :w], in_=sbs[hh][:, :w], func=Exp
                        )
                        pband = work.tile([128, 1024], bf16, tag="pb", name="pb", bufs=4)
                        geng = nc.vector if w >= 768 else nc.gpsimd
                        geng.tensor_mul(
                            pband[:, :w],
                            pex[:, :w],
                            g_sb[2 * p + hh][:, gs : gs + w],
                        )
                        pbs.append(pband)

                    def emit_pv(kb=kb, qq=qq, w=w, pbs=pbs, mixs=mixs, p=p,
                                qbase=qbase, qhalf=qhalf):
                        for hh in range(2):
                            h = 2 * p + hh
                            for c in range(2):
                                lo = max(qq, qbase + 512 * c)
                                hi = min(qq + w, qbase + 512 * (c + 1))
                                if lo >= hi:
                                    continue
                                nc.tensor.matmul(
                                    mixs[hh][:65, lo - qbase : hi - qbase],
                                    lhsT=v_sb[kb][:, h * 65 : (h + 1) * 65],
                                    rhs=pbs[hh][:, lo - qq : hi - qq],
                                    start=(kb == 0),
                                    stop=(kb == last_kb[qhalf][c]),
                                )

                    pend.setdefault(slot + 1, []).append(emit_pv)
                    if qhalf == 0 and filler:
                        filler.pop(0)()
                    slot += 1
                for fn in pend.pop(slot, []):
                    fn()
                is_last = (qhalf, p) == groups[-1]
                chains = [
                    dance(mixs, p, qbase, hh, "act" if (is_last and hh == 1) else None)
                    for hh in range(2)
                ]
                for hh in range(2):
                    pend.setdefault(slot + 1, []).append(
                        lambda ch=chains[hh]: pend.setdefault(slot + 2, []).append(
                            ch()
                        )
                    )
            while pend:
                si = min(pend)
                for fn in pend.pop(si):
                    fn()

            # ---------------- Phase C: output projection ----------------
            for qb in range(KB):
                outproj(qb, ("sb", "sb", "mxA", "mxB")[qb % 4])
    nc.finalize()
    return nc


def host_prep(x, qkv, out_w, rpe_bias):
    """Build per-core input shards (all host work is O(N*S) or a transpose)."""
    x = np.asarray(x, np.float32)
    qkv = np.asarray(qkv, np.float32)
    out_w = np.asarray(out_w, np.float32)
    rpe_bias = np.asarray(rpe_bias, np.float32)

    xT = [np.ascontiguousarray(x[b].T).astype(BF16) for b in range(B)]  # [HID,S]

    shards = []
    for hg in range(4):
        hs = slice(hg * 4, hg * 4 + 4)
        wq = qkv[:, 0, hs, :].reshape(HID, 256) * (D ** -0.5)
        wk = qkv[:, 1, hs, :].reshape(HID, 256)
        w_qk = np.concatenate([wq, wk], axis=1).astype(BF16)

        w_v = np.zeros((HID, 260), np.float32)
        for i in range(4):
            w_v[:, i * 65 : i * 65 + 64] = qkv[:, 2, hg * 4 + i, :]
        w_v = w_v.astype(BF16)

        g = np.zeros((HPC, 128, 512 + S), np.float32)
        idx = np.arange(512 + S)[None, :] - 512 - np.arange(128)[:, None]
        valid = (idx >= 0) & (idx < S)
        for i in range(4):
            cum = np.cumsum(rpe_bias[hg * 4 + i])
            gh = np.exp(cum)
            g[i] = np.where(valid, gh[np.clip(idx, 0, S - 1)], 0.0)
        g = g.astype(BF16)

        w_o = out_w[hs].reshape(256, HID).astype(BF16)
        shards.append((w_qk, w_v, g, w_o))

    in_maps = []
    for c in range(NCORES):
        b, hg = c // 4, c % 4
        w_qk, w_v, g, w_o = shards[hg]
        in_maps.append(
            {"x_t": xT[b], "w_qk": w_qk, "w_v": w_v, "g_ext": g, "w_out": w_o}
        )
    return in_maps


def run(in_maps, trace=False):
    from concourse.bass_utils import run_bass_kernel_spmd

    if "nc" not in _CACHE:
        _CACHE["nc"] = build_nc()
    nc = _CACHE["nc"]
    res = run_bass_kernel_spmd(nc, in_maps, core_ids=list(range(NCORES)), trace=trace)
    return res


def kernel(x, qkv, out_w, rpe_bias):
    in_maps = host_prep(x, qkv, out_w, rpe_bias)
    res = run(in_maps)
    parts = [np.asarray(res.results[c]["out"], np.float32) for c in range(NCORES)]
    out = np.stack(
        [
            parts[0] + parts[1] + parts[2] + parts[3],
            parts[4] + parts[5] + parts[6] + parts[7],
        ]
    ).astype(np.float32)
    return out


if __name__ == "__main__":
    nc = build_nc()
    print("built ok")
